# revision 30
# baseline (speedup 1.0000x reference)
"""Trainium2 Bass kernel for nn_Attention_56530359550323.

Full-input contract: kernel(**inputs) takes the unsharded inputs and returns
the full [4, 2048, 4096] float32 output.

Sharding: 8 cores = 4 batches (data-parallel) x 2 head-groups
(tensor-parallel over the 4 query heads; the single kv head is replicated).
Each core computes a partial output-projection [4096, 2048] (transposed);
the host sums the two partials per batch ("all-reduce after wo") and
transposes back.

Device algorithm (feature-major / transposed; all matmuls f16 with a
512-wide moving operand; one ACT table set natural_log_exp_and_others —
exp/ln/square/copy — so there are no mid-kernel table swaps):

  software pipeline over the 4 s-blocks:
    proj(sb):  qT/kT/vT = W^T @ xT accumulated over 32 d-chunks into 4 PSUM
               banks; weight DMAs are interleaved per d-group with the sb0
               x tiles so the first matmul starts ~1MB into the DMA stream.
    epi(sb):   emitted one block late (during proj(sb+1)) so its DVE rope
               never stalls the PE: RoPE in f16 (2x DVE), sumsq via a
               ones[128,128] matmul (row-sum pre-broadcast across
               partitions), rsqrt as exp(-0.5*ln(m+eps)) on ACT, v
               transposed to natural layout with PE transposes.
    attn(qb):  scoresT = kT_chunk^T @ qT per 128-kv chunk, exp on ACT
               (scores bounded ~5 post-qk-norm, no max subtraction),
               causal masking on the 4 diagonal chunks, PV accumulation,
               denominator = DVE f16 chunk-sum + one ones[128,128] matmul
               (broadcast row-sum), 1/d = exp(-ln d) on ACT.
    outproj(qb): partial projection sum_h wo[h,cc]^T @ attnT_h, interleaved
               into the next q-block's attention chunks to keep the PE
               dense; f16 partials DMAed out (host sums in fp32).
"""

import os
import sys
from collections import deque

import numpy as np

if "/opt/trn_rl_repo" not in sys.path:
    sys.path.insert(0, "/opt/trn_rl_repo")

import concourse.bass as bass
import concourse.mybir as mybir
import concourse.tile as tile
from concourse import bacc, bass_utils
from concourse import hw_specs as _hw_specs

# The act-table-load pass assigns each activation function the FIRST table
# set containing it: Exp -> exp_and_others, Ln -> natural_log. This kernel
# interleaves Ln with Exp throughout (rsqrt/reciprocal computed as
# exp(-a*ln(x))), which would reload the ACT tables on every alternation
# (~35 loads x 1.3us). natural_log_exp_and_others contains every function
# used here (exp, ln, copy, identity, square), so steer the pass to it by
# blanking the other sets' function lists (positions preserved: the emitted
# act_func_set_id indexes act_info.json's set order).
_ACT_SET = "natural_log_exp_and_others"


def _single_set_tables(arch):
    tabs = _hw_specs.get_activation_tables(arch)
    return {k: (v if k == _ACT_SET else set()) for k, v in tabs.items()}


bacc.get_activation_tables = _single_set_tables

# ---- problem constants (hardcoded per contract) ----
B, S, D = 4, 2048, 4096
HEAD_DIM = 128
N_HEADS = 4            # local q heads in the reference module
N_KV = 1
ROPE_THETA = 500000.0
EPS = 1e-6
FLOOR_SCALE = 8192.0
ATTN_SCALE = 0.1

P = 128                # partitions
SB = 512               # s-block (q-block) size
NSB = S // SB          # 4
ND = D // P            # 32 contraction chunks for projections
NKCH = S // P          # 16 kv chunks
NCC = D // P           # 32 output column chunks
HG = 2                 # heads per group (tensor-parallel degree 2)

f32 = mybir.dt.float32
f16 = mybir.dt.float16
u8 = mybir.dt.uint8

# GPSIMD copies that cast f32 PSUM -> f16 SBUF; disable to route them to
# ACT/DVE if the Pool engine can't convert dtypes.
GPSIMD_CAST = os.environ.get("KERNEL_GPSIMD_CAST", "1") == "1"

_BUILD_CACHE = {}


def build_bass():
    key = GPSIMD_CAST
    if key in _BUILD_CACHE:
        return _BUILD_CACHE[key]

    nc = bacc.Bacc("TRN2", target_bir_lowering=False, debug=False)

    # all big tensors arrive pre-tiled host-side so every DMA is a
    # contiguous per-partition read
    xT_d = nc.dram_tensor("xT", (NSB, 8, P, 4, SB), f16, kind="ExternalInput").ap()
    wq_d = nc.dram_tensor("wq_g", (8, P, 4, HG * HEAD_DIM), f16, kind="ExternalInput").ap()
    wk_d = nc.dram_tensor("wk", (8, P, 4, HEAD_DIM), f16, kind="ExternalInput").ap()
    wv_d = nc.dram_tensor("wv", (8, P, 4, HEAD_DIM), f16, kind="ExternalInput").ap()
    wo_d = nc.dram_tensor("wo_g", (P, HG, NCC, P), f16, kind="ExternalInput").ap()
    cs_d = nc.dram_tensor("csT", (64, S), f16, kind="ExternalInput").ap()
    sn_d = nc.dram_tensor("snT", (64, S), f16, kind="ExternalInput").ap()
    qs_d = nc.dram_tensor("qscale", (1, S), f16, kind="ExternalInput").ap()
    out_d = nc.dram_tensor("outT", (NCC, NSB, P, SB), f16, kind="ExternalOutput").ap()

    # masks for the 4 diagonal chunks of a 512-q block: 1 => future (kill)
    masks_np = np.zeros((P, 4, SB), np.uint8)
    for c in range(4):
        kp = c * P + np.arange(P)[:, None]
        qf = np.arange(SB)[None, :]
        masks_np[:, c, :] = (kp > qf).astype(np.uint8)
    masks_d = nc.inline_tensor(masks_np, name="cmasks")
    ident_d = nc.inline_tensor(np.eye(P, dtype=np.float16), name="ident")

    Exp = mybir.ActivationFunctionType.Exp
    Ln = mybir.ActivationFunctionType.Ln
    Copy = mybir.ActivationFunctionType.Copy

    with tile.TileContext(nc) as tc:
        top = tc.tile_pool(name="consts", bufs=1)
        cpool = top.__enter__()
        qk_cm = tc.tile_pool(name="qkv", bufs=1)
        qkpool = qk_cm.__enter__()
        at_cm = tc.tile_pool(name="attn", bufs=1)
        atpool = at_cm.__enter__()

        # ---- const tiles (memsets are instant; DMAs ordered for startup) ----
        ones128_t = cpool.tile([P, P], f16)
        nc.vector.memset(ones128_t, 1.0)
        onesrow_t = cpool.tile([1, P], f16)
        nc.vector.memset(onesrow_t, 1.0)
        zero_t = cpool.tile([P, SB], f16)
        nc.vector.memset(zero_t, 0.0)
        epsb_t = cpool.tile([P, 1], f32)
        nc.vector.memset(epsb_t, float(EPS))

        # weight/x pools
        w_cm = tc.tile_pool(name="projw", bufs=1)
        wpool = w_cm.__enter__()
        x_cm = tc.tile_pool(name="xstream", bufs=8)
        xpool = x_cm.__enter__()
        e_cm = tc.tile_pool(name="ep1", bufs=2)
        epool = e_cm.__enter__()
        t_cm = tc.tile_pool(name="ropetmp", bufs=2)
        tpool = t_cm.__enter__()
        ex_cm = tc.tile_pool(name="exps", bufs=12)
        expool = ex_cm.__enter__()
        o3_cm = tc.tile_pool(name="oc", bufs=16)
        o3pool = o3_cm.__enter__()

        wq_t = wpool.tile([P, ND, HG * HEAD_DIM], f16)
        wk_t = wpool.tile([P, ND, HEAD_DIM], f16)
        wv_t = wpool.tile([P, ND, HEAD_DIM], f16)

        # qscale first (tiny, needed by epi0)
        qs_t = cpool.tile([1, S], f16)
        nc.sync.dma_start(qs_t, qs_d)

        # all input DMAs share the sync HWDGE ring: FIFO order = service
        # order, so emission position is the priority knob.
        cs_t = cpool.tile([P, S], f16)
        sn_t = cpool.tile([P, S], f16)
        masks_t = cpool.tile([P, 4, SB], u8)
        ident_t = cpool.tile([P, P], f16)
        wo_t = cpool.tile([P, HG, NCC, P], f16)

        # critical-path DMA interleave: per d-group weights then the sb0
        # x tile, so the first matmuls wait on ~1MB instead of ~6.5MB.
        # cs/sn slot in after dg5 (the rope for sb0 needs them right when
        # proj(1) starts); ident after dg6 (epi(0) transposes).
        xtiles0 = []
        for dg in range(8):
            dsl = slice(dg * 4, (dg + 1) * 4)
            xt = xpool.tile([P, 4, SB], f16, tag="xt", name=f"xt0_{dg}")
            if dg == 0:
                # fine-grained first tiles: the very first matmul waits on
                # ~0.25MB instead of the whole d-group
                for c in range(4):
                    nc.sync.dma_start(wq_t[:, c, :], wq_d[0, :, c])
                    nc.sync.dma_start(wk_t[:, c, :], wk_d[0, :, c])
                    nc.sync.dma_start(wv_t[:, c, :], wv_d[0, :, c])
                    nc.sync.dma_start(xt[:, c, :], xT_d[0, 0, :, c])
            else:
                nc.sync.dma_start(wq_t[:, dsl, :], wq_d[dg])
                nc.sync.dma_start(wk_t[:, dsl, :], wk_d[dg])
                nc.sync.dma_start(wv_t[:, dsl, :], wv_d[dg])
                nc.sync.dma_start(xt, xT_d[0, dg])
            xtiles0.append(xt)
            if dg == 5:
                nc.sync.dma_start(cs_t[0:64, :], cs_d)
                nc.sync.dma_start(cs_t[64:128, :], cs_d)
                nc.sync.dma_start(sn_t[0:64, :], sn_d)
                nc.sync.dma_start(sn_t[64:128, :], sn_d)
            elif dg == 6:
                nc.sync.dma_start(ident_t, ident_d.ap())
        nc.sync.dma_start(masks_t, masks_d.ap())

        # cross-phase SBUF handoff tiles
        qT_t = qkpool.tile([P, HG, S], f16)       # normed+roped+scaled qT
        kT_t = qkpool.tile([P, S], f16)           # normed+roped kT
        vnat_t = qkpool.tile([P, NKCH, P], f16)   # v in natural [s, hd] tiles
        attnT_t = atpool.tile([P, HG, S], f16)

        # ---------------- PSUM pools ----------------
        # pEpi outlives pProj (pools release LIFO): pEpi first, pProj on top.
        pepi_cm = tc.tile_pool(name="pepi", bufs=3, space="PSUM")
        pEpi = pepi_cm.__enter__()
        pproj_cm = tc.tile_pool(name="pproj", bufs=1, space="PSUM")
        pProj = pproj_cm.__enter__()

        # ---------------- phase 1: projections + epilogues ----------------
        def proj(sb, mid=None):
            """128 projection matmuls for one s-block; returns psum tiles.
            `mid` (the previous block's epilogue) is emitted after dg3 so
            its PE ops sit mid-stream and its ACT ladder finishes well
            before this block ends."""
            q_ps = [
                pProj.tile([P, SB], f32, tag=f"q{h}", name=f"qps{h}")
                for h in range(HG)
            ]
            k_ps = pProj.tile([P, SB], f32, tag="k")
            v_ps = pProj.tile([P, SB], f32, tag="v")
            for dg in range(8):
                if sb == 0:
                    xt = xtiles0[dg]
                else:
                    xt = xpool.tile([P, 4, SB], f16, tag="xt", name=f"xt{sb}_{dg}")
                    nc.sync.dma_start(xt, xT_d[sb, dg])
                for c in range(4):
                    d = dg * 4 + c
                    st, sp = (d == 0), (d == ND - 1)
                    rhs = xt[:, c, :]
                    for h in range(HG):
                        nc.tensor.matmul(
                            q_ps[h], wq_t[:, d, h * P:(h + 1) * P], rhs,
                            start=st, stop=sp,
                        )
                    nc.tensor.matmul(k_ps, wk_t[:, d, :], rhs, start=st, stop=sp)
                    nc.tensor.matmul(v_ps, wv_t[:, d, :], rhs, start=st, stop=sp)
                if dg == 3 and mid is not None:
                    mid()
            return q_ps, k_ps, v_ps

        def copies(sb, q_ps, k_ps, v_ps):
            """Free accumulator banks; engine split so the next s-block's
            first matmuls aren't serialized behind one engine."""
            qc0 = epool.tile([P, SB], f16, tag="qc0", bufs=NSB, name="qc0")
            nc.scalar.copy(qc0, q_ps[0])
            qc1 = epool.tile([P, SB], f16, tag="qc1", bufs=NSB, name="qc1")
            nc.vector.tensor_copy(qc1, q_ps[1])
            kc = epool.tile([P, SB], f16, tag="kc", bufs=NSB)
            nc.scalar.copy(kc, k_ps)
            vc = epool.tile([P, SB], f16, tag="vc", bufs=NSB)
            nc.vector.tensor_copy(vc, v_ps)
            return [qc0, qc1], kc, vc

        def rope_sumsq(srcc, sb, ppool, ptag):
            """Pass A of the epilogue (DVE + one PE matmul, no ACT): RoPE in
            f16 (2x DVE), then sumsq broadcast across partitions via a
            ones[128,128] matmul. Returns (rope, ss_ps)."""
            ss = slice(sb * SB, (sb + 1) * SB)
            rope = epool.tile([P, SB], f16, tag="rope", bufs=3)
            ta = tpool.tile([64, SB], f16, tag="ta")
            tb = tpool.tile([64, SB], f16, tag="tb")
            te, to = srcc[0:64, :], srcc[64:128, :]
            # top half: te*cos - to*sin
            nc.vector.tensor_mul(ta, te, cs_t[0:64, ss])
            nc.vector.tensor_mul(tb, to, sn_t[64:128, ss])
            nc.vector.tensor_sub(rope[0:64, :], ta, tb)
            # bottom half: to*cos + te*sin
            tc_ = tpool.tile([64, SB], f16, tag="tc")
            td = tpool.tile([64, SB], f16, tag="td")
            nc.vector.tensor_mul(tc_, to, cs_t[64:128, ss])
            nc.vector.tensor_mul(td, te, sn_t[0:64, ss])
            nc.vector.tensor_add(rope[64:128, :], tc_, td)
            sq = epool.tile([P, SB], f16, tag="sq")
            nc.vector.tensor_mul(sq, rope, rope)
            ss_ps = ppool.tile([P, SB], f32, tag=ptag, name="ssps")
            nc.tensor.matmul(ss_ps, ones128_t, sq, start=True, stop=True)
            return rope, ss_ps

        def norm_apply(dst, rope, ss_ps, qsbc):
            """Pass B (ACT ladder + DVE muls): factor = (mean+eps)^-0.5 as
            exp(-0.5*ln(.)), times the broadcast qscale for q heads."""
            lnm = epool.tile([P, SB], f32, tag="lnm")
            nc.scalar.activation(lnm, ss_ps, Ln, bias=epsb_t[:], scale=1.0 / HEAD_DIM)
            fac = epool.tile([P, SB], f16, tag="fac")
            nc.scalar.activation(fac, lnm, Exp, scale=-0.5)
            if qsbc is not None:
                facq = epool.tile([P, SB], f16, tag="facq")
                nc.vector.tensor_mul(facq, fac, qsbc)
                fac = facq
            nc.vector.tensor_mul(dst, rope, fac)

        def epi_v(sb, vc, ppool, ptag):
            for t in range(4):
                tp_ps = ppool.tile([P, P], f16, tag=ptag, name="tp")
                nc.tensor.transpose(tp_ps, vc[:, t * P:(t + 1) * P], ident_t[:])
                nc.vector.tensor_copy(vnat_t[:, sb * 4 + t, :], tp_ps)

        def epi_qsbc(sb, ppool, ptag):
            # broadcast qscale over partitions (shared by both q heads)
            ss = slice(sb * SB, (sb + 1) * SB)
            bcq_ps = ppool.tile([P, SB], f32, tag=ptag, name="bcq")
            nc.tensor.matmul(bcq_ps, onesrow_t, qs_t[:, ss], start=True, stop=True)
            qsbc = epool.tile([P, SB], f16, tag="qsbc")
            nc.scalar.copy(qsbc, bcq_ps)
            return qsbc

        def epi_head(sb, srcc, dst, qsbc, ppool, ptag):
            norm_apply(dst, *rope_sumsq(srcc, sb, ppool, ptag), qsbc)

        def epi(sb, qcopies, kc, vc):
            """PE-light epilogue: transposes and ssbc matmuls first (their
            deps are DVE-only), the ACT ln/exp ladder after — so the PE
            stream never waits on ACT."""
            ss = slice(sb * SB, (sb + 1) * SB)
            epi_v(sb, vc, pEpi, "epi")
            qsbc = epi_qsbc(sb, pEpi, "epi")
            passa = [rope_sumsq(qcopies[h], sb, pEpi, "epi") for h in range(HG)]
            passa.append(rope_sumsq(kc, sb, pEpi, "epi"))
            for h in range(HG):
                norm_apply(qT_t[:, h, ss], *passa[h], qsbc)
            norm_apply(kT_t[:, ss], *passa[HG], None)

        acc = []
        for sb in range(NSB):
            mid = None
            if sb >= 1:
                mid = (lambda s=sb - 1: epi(s, *acc[s]))
            ps = proj(sb, mid=mid)
            acc.append(copies(sb, *ps))
            if sb == 3:
                # wo queues behind sb3's x tiles; needed ~50us later
                nc.sync.dma_start(wo_t, wo_d)
        # note: epi(3) is sliced into the attention region below so its
        # DVE rope bursts interleave with attn PE work (it allocates PSUM
        # from the attention pools, letting both phase-1 pools close here).

        # free all 7 phase-1 banks (LIFO), open attention pools
        pproj_cm.__exit__(None, None, None)
        pepi_cm.__exit__(None, None, None)
        # pSc holds 2-bank [P, 2*SB] score tiles: two kv chunks share one
        # ACT exp call (FD=1024), halving the per-op fixed cost.
        psc_cm = tc.tile_pool(name="psc", bufs=2, space="PSUM")
        pSc = psc_cm.__enter__()
        ppv_cm = tc.tile_pool(name="ppv", bufs=2, space="PSUM")
        pPv = ppv_cm.__enter__()
        po_cm = tc.tile_pool(name="po", bufs=2, space="PSUM")
        pO = po_cm.__enter__()

        # ---------------- phase 2/3: attention + out-projection ----------------
        pending = deque()     # outproj cc units not yet emitted
        ncopy = [0]           # round-robin counter for outproj copies

        def outproj_unit():
            cc, qb = pending.popleft()
            qsl = slice(qb * SB, (qb + 1) * SB)
            o_ps = pO.tile([P, SB], f32, tag="o", name="ops")
            for h in range(HG):
                nc.tensor.matmul(
                    o_ps, wo_t[:, h, cc, :], attnT_t[:, h, qsl],
                    start=(h == 0), stop=(h == HG - 1),
                )
            o_sb = o3pool.tile([P, SB], f16, tag="oc")
            if ncopy[0] % 2 == 0:
                nc.vector.tensor_copy(o_sb, o_ps)
            else:
                nc.scalar.copy(o_sb, o_ps)
            ncopy[0] += 1
            nc.sync.dma_start(out_d[cc, qb], o_sb)

        def attn_group(qb, h, drain_plan, prev_fin):
            """One (q-block, head) softmax-attention accumulation.
            drain_plan[c] = number of pending outproj units to emit after
            chunk c (keeps the PE dense while ACT does the exps). The
            previous group's finalize is emitted two chunks in, so its
            denominator chain never stalls the PE. Returns this group's
            finalize closure."""
            qsl = slice(qb * SB, (qb + 1) * SB)
            nch = 4 * qb + 4
            pv_ps = pPv.tile([P, SB], f32, tag="pv")
            qt = qT_t[:, h, qsl]
            state = {"es": None}
            for pr in range(nch // 2):
                sc2 = pSc.tile([P, 2, SB], f32, tag="sc2", name="sc2")
                for j in (0, 1):
                    c = 2 * pr + j
                    nc.tensor.matmul(sc2[:, j, :], kT_t[:, c * P:(c + 1) * P],
                                     qt, start=True, stop=True)
                e2 = expool.tile([P, 2, SB], f16, tag="exp", bufs=6)
                nc.scalar.activation(e2, sc2, Exp)
                for j in (0, 1):
                    c = 2 * pr + j
                    e_sb = e2[:, j, :]
                    if c >= 4 * qb:
                        # diagonal chunk: q < t*128 is fully future (memset),
                        # the next 128 q columns are triangular (predicated)
                        t = c - 4 * qb
                        if t > 0:
                            nc.vector.memset(e_sb[:, 0:t * P], 0.0)
                        tsl = slice(t * P, (t + 1) * P)
                        nc.vector.copy_predicated(
                            e_sb[:, tsl], masks_t[:, t, tsl], zero_t[:, tsl]
                        )
                    st, sp = (c == 0), (c == nch - 1)
                    nc.tensor.matmul(pv_ps, vnat_t[:, c, :], e_sb,
                                     start=st, stop=sp)
                    # f16 chunk-sum of exps (max rowsum ~3.7e3, in range)
                    if state["es"] is None:
                        state["es"] = e_sb
                    else:
                        es = expool.tile([P, SB], f16, tag="es", bufs=3,
                                         name="es")
                        nc.vector.tensor_add(es, state["es"], e_sb)
                        state["es"] = es
                if pr == 0 and prev_fin is not None:
                    prev_fin()
                for _ in range(drain_plan[pr]):
                    if pending:
                        outproj_unit()

            def finalize():
                # denominator, pre-broadcast over partitions; 1/d = exp(-ln d)
                dbc_ps = pSc.tile([P, SB], f32, tag="sc2", name="dbc")
                nc.tensor.matmul(dbc_ps, ones128_t, state["es"],
                                 start=True, stop=True)
                lnd = epool.tile([P, SB], f32, tag="lnd")
                nc.scalar.activation(lnd, dbc_ps, Ln)
                rec = epool.tile([P, SB], f16, tag="rec")
                nc.scalar.activation(rec, lnd, Exp, scale=-1.0)
                pvs = epool.tile([P, SB], f16, tag="pvs")
                nc.vector.tensor_copy(pvs, pv_ps)
                nc.vector.tensor_mul(attnT_t[:, h, qsl], pvs, rec)

            return finalize

        def make_plan(qb, h):
            """Spread pending outproj units evenly over this group's chunks,
            but skip the first chunks of h==0 (attnT for the previous block
            finalizes a couple of chunks into the group)."""
            npair = (4 * qb + 4) // 2
            plan = [0] * npair
            if not pending:
                return plan
            navail = len(pending)
            # both heads' pairs share the drain; emit this group's share
            share = navail if h == 1 else navail - navail // 2
            start = min(2, npair - 1) if h == 0 else 0
            slots = npair - start
            for i in range(share):
                plan[start + (i * slots) // share] += 1
            return plan

        # epi(3) slices, spread across group boundaries so its DVE bursts
        # never block a group's pipeline; only attn(qb3) consumes them.
        q3 = {}

        def epi3_slice(qb, h):
            ss3 = slice(3 * SB, 4 * SB)
            if qb == 0 and h == 1:
                epi_v(3, acc[3][2], pSc, "sc2")
                q3["qsbc"] = epi_qsbc(3, pSc, "sc2")
            elif qb == 1 and h == 0:
                epi_head(3, acc[3][0][0], qT_t[:, 0, ss3], q3["qsbc"], pSc, "sc2")
            elif qb == 1 and h == 1:
                epi_head(3, acc[3][0][1], qT_t[:, 1, ss3], q3["qsbc"], pSc, "sc2")
            elif qb == 2 and h == 0:
                epi_head(3, acc[3][1], kT_t[:, ss3], None, pSc, "sc2")

        prev_fin = None
        for qb in range(NSB):
            for h in range(HG):
                prev_fin = attn_group(qb, h, make_plan(qb, h), prev_fin)
                epi3_slice(qb, h)
            pending.extend((cc, qb) for cc in range(NCC))
        prev_fin()
        while pending:
            outproj_unit()

        for cm in (po_cm, ppv_cm, psc_cm, o3_cm, ex_cm, t_cm,
                   e_cm, x_cm, w_cm, at_cm, qk_cm, top):
            cm.__exit__(None, None, None)

    nc.compile()
    _BUILD_CACHE[key] = nc
    return nc


def _host_prep(x, positions, wq, wk, wv, wo):
    """Returns per-core input maps."""
    pos_f = positions.astype(np.float32)
    inv_freq = (
        1.0
        / (ROPE_THETA ** (np.arange(0, HEAD_DIM, 2, dtype=np.float32) / HEAD_DIM))
    ).astype(np.float32)
    ang = pos_f[:, None] * inv_freq[None, :]        # [S, 64]
    csT = np.ascontiguousarray(np.cos(ang).T).astype(np.float16)  # [64, S]
    snT = np.ascontiguousarray(np.sin(ang).T).astype(np.float16)  # [64, S]
    attn_scales = (
        np.log(np.floor((pos_f + 1.0) / FLOOR_SCALE) + 1.0) * ATTN_SCALE + 1.0
    )
    qscale = (attn_scales / np.sqrt(np.float32(HEAD_DIM))).astype(np.float16)[None, :]

    # rotate-half permutation of q/k feature dims (per head), folded into
    # the projection weight columns: permuted feature j<64 <- 2j, j>=64 <- 2(j-64)+1
    perm = np.concatenate([np.arange(0, HEAD_DIM, 2), np.arange(1, HEAD_DIM, 2)])
    wq_p = wq.reshape(D, N_HEADS, HEAD_DIM)[:, :, perm].reshape(D, N_HEADS * HEAD_DIM)
    wk_p = wk[:, perm]

    def tile_x(xT):
        # [D, S] -> [sb, dg, p, c, s]
        return np.ascontiguousarray(
            xT.reshape(8, 4, P, NSB, SB).transpose(3, 0, 2, 1, 4)
        )

    def tile_w(w):
        # [D, m] -> [dg, p, c, m]
        m = w.shape[1]
        return np.ascontiguousarray(
            w.reshape(8, 4, P, m).transpose(0, 2, 1, 3)
        )

    def tile_wo(wg):
        # [256, D] -> [p, hh, cc, q]
        return np.ascontiguousarray(
            wg.reshape(HG, P, NCC, P).transpose(1, 0, 2, 3)
        )

    in_maps = []
    for core in range(8):
        b, g = core // 2, core % 2
        xT = np.ascontiguousarray(x[b].T).astype(np.float16, copy=False)
        in_maps.append(
            {
                "xT": tile_x(xT),
                "wq_g": tile_w(
                    wq_p[:, g * HG * HEAD_DIM:(g + 1) * HG * HEAD_DIM].astype(np.float16)
                ),
                "wk": tile_w(wk_p.astype(np.float16)),
                "wv": tile_w(wv.astype(np.float16)),
                "wo_g": tile_wo(
                    wo[g * HG * HEAD_DIM:(g + 1) * HG * HEAD_DIM, :].astype(np.float16)
                ),
                "csT": csT,
                "snT": snT,
                "qscale": qscale,
            }
        )
    return in_maps


def kernel(x, positions, wq, wk, wv, wo, _trace=False, _trace_kwargs=None):
    x = np.asarray(x, np.float32)
    positions = np.asarray(positions)
    wq = np.asarray(wq, np.float32)
    wk = np.asarray(wk, np.float32)
    wv = np.asarray(wv, np.float32)
    wo = np.asarray(wo, np.float32)

    nc = build_bass()
    in_maps = _host_prep(x, positions, wq, wk, wv, wo)
    res = bass_utils.run_bass_kernel_spmd(
        nc, in_maps, core_ids=list(range(8)), trace=_trace,
        **(_trace_kwargs or {}),
    )
    kernel.last_results = res

    out = np.empty((B, S, D), np.float32)
    for b in range(B):
        pa = res.results[2 * b]["outT"].astype(np.float32)
        pb = res.results[2 * b + 1]["outT"].astype(np.float32)
        full = (pa + pb).transpose(0, 2, 1, 3).reshape(D, S)
        out[b] = full.T
    return out
